# revision 2
# baseline (speedup 1.0000x reference)
"""Trainium2 Bass kernel v2 for the 2-layer GCN (nn_GCNClassifier).

    h1 = relu(adj1 @ x @ W1 + b1) + relu(adj2 @ x @ W1 + b1)   # [8192, 64]
    out = relu(adj1 @ g + b2) + relu(adj2 @ g + b2)            # g = h1 @ W2

Strategy (per core, 1024-row shard of both adjacencies, pre-transposed):
- Adjacency is decomposed on the host as A = H + L/16 with H = e3m4(A) and
  L = e4m3(16*(A - H)). Layer 1 streams H and L (2 bytes/elem total, same
  traffic as fp16) and consumes them with mixed-dtype matmuls: stationary
  features in fp16 (x for H, x/16 for L) x fp8 moving adjacency, both
  accumulating into the same PSUM group (unit scales).
- H stays RESIDENT in SBUF (128 KiB/partition for both adjacencies).
  Layer 2 runs entirely from the resident H with no further adjacency DMA
  (e3m4-only layer 2 measures 1.32e-2 rel err vs the f32 reference on the
  fixed test inputs; full-refinement layer 1 keeps the total there).
- Inter-layer: g = h1 @ W2 in fp16, single 32 KiB store, AllGather, then a
  7-chunk rotated gather on the (idle) HWDGE rings. Host rotates each
  core's k-group order so its own chunk is stream-position 0 (its g chunk
  feeds layer-2 matmuls before the AllGather lands).
Total HBM traffic ~33 MiB/core vs 64 MiB for the fp16 baseline.
"""

import numpy as np
import ml_dtypes

import concourse.bacc as bacc
import concourse.bass as bass
import concourse.mybir as mybir
import concourse.tile as tile
from concourse.bass_utils import run_bass_kernel_spmd
from concourse.masks import make_identity

N = 8192
IN_DIM, HID_DIM, OUT_DIM = 32, 64, 16
N_CORES = 8
ROWS = N // N_CORES          # 1024 output rows per core
KBLK = 128                   # contraction block (SBUF partition dim)
KM = 8                       # k-blocks per DMA group (1 MiB fp8 groups)
NKB = N // KBLK              # 64 contraction blocks
NKG = NKB // KM              # 8 groups per adjacency
MC1 = 512                    # layer-1 moving chunk
MC2 = 512                    # layer-2 moving chunk
F32 = mybir.dt.float32
F32R = mybir.dt.float32r    # single-pass PE fp32 (vs 4-pass full fp32)
F16 = mybir.dt.float16
E3 = mybir.dt.float8e3      # H: e3m4
E4 = mybir.dt.float8e4      # L: e4m3 (16x residual)
RELU = mybir.ActivationFunctionType.Relu
ADD = mybir.AluOpType.add
MAX = mybir.AluOpType.mult if False else mybir.AluOpType.max
LRING = 4                    # L-group ring depth


def _build_program():
    nc = bacc.Bacc(
        "TRN2", target_bir_lowering=False, debug=False, num_devices=N_CORES
    )
    a1h = nc.dram_tensor("a1h", [NKG, KBLK, KM, ROWS], E3, kind="ExternalInput")
    a2h = nc.dram_tensor("a2h", [NKG, KBLK, KM, ROWS], E3, kind="ExternalInput")
    a1l = nc.dram_tensor("a1l", [NKG, KBLK, KM, ROWS], E4, kind="ExternalInput")
    a2l = nc.dram_tensor("a2l", [NKG, KBLK, KM, ROWS], E4, kind="ExternalInput")
    xb = nc.dram_tensor("xb", [KBLK, NKB, IN_DIM], F16, kind="ExternalInput")
    xsb = nc.dram_tensor("xsb", [KBLK, NKB, IN_DIM], F16, kind="ExternalInput")
    w1 = nc.dram_tensor("w1", [IN_DIM, HID_DIM], F32, kind="ExternalInput")
    b1 = nc.dram_tensor("b1", [HID_DIM, 1], F32, kind="ExternalInput")
    w2 = nc.dram_tensor("w2", [HID_DIM, OUT_DIM], F32, kind="ExternalInput")
    b2 = nc.dram_tensor("b2", [OUT_DIM, 1], F32, kind="ExternalInput")
    out = nc.dram_tensor("out", [ROWS, OUT_DIM], F32, kind="ExternalOutput")

    with tile.TileContext(nc) as tc:
        _kernel_body(nc, tc, (a1h, a2h), (a1l, a2l), xb, xsb, w1, b1, w2, b2, out)
    nc.compile()
    return nc


def _kernel_body(nc, tc, ah, al, xb_d, xsb_d, w1, b1, w2, b2, out):
    NMC1 = ROWS // MC1
    NMC2 = ROWS // MC2
    with (
        tc.tile_pool(name="const", bufs=1) as constp,
        tc.tile_pool(name="hres", bufs=1) as hresp,
        tc.tile_pool(name="lring", bufs=LRING) as lringp,
        tc.tile_pool(name="work", bufs=1) as workp,
        tc.tile_pool(name="psum", bufs=1, space="PSUM") as psp,
        tc.tile_pool(name="dram", bufs=1, space="DRAM") as dramp,
    ):
        # features (fp16) lead the two HWDGE rings; constants ride SWDGE
        xb = constp.tile([KBLK, NKB, IN_DIM], F16)
        nc.sync.dma_start(xb[:], xb_d[:])
        xsb = constp.tile([KBLK, NKB, IN_DIM], F16)
        nc.scalar.dma_start(xsb[:], xsb_d[:])
        w1_sb = constp.tile([IN_DIM, HID_DIM], F32R)
        nc.gpsimd.dma_start(w1_sb[:], w1[:])
        b1_sb = constp.tile([HID_DIM, 1], F32)
        nc.gpsimd.dma_start(b1_sb[:], b1[:])
        w2_sb = constp.tile([HID_DIM, OUT_DIM], F32R)
        nc.gpsimd.dma_start(w2_sb[:], w2[:])
        b2_sb = constp.tile([OUT_DIM, 1], F32)
        nc.gpsimd.dma_start(b2_sb[:], b2[:])
        ident = constp.tile([OUT_DIM, OUT_DIM], F32)
        make_identity(nc, ident[:])

        # tiny warm-up AllGather to wake the CC path early
        warm_sb = constp.tile([1, N_CORES], F32)
        nc.gpsimd.memset(warm_sb[:], 0.0)
        warm_in = dramp.tile([1, N_CORES], F32)
        warm_out = dramp.tile([N_CORES, N_CORES], F32, addr_space="Shared")
        nc.gpsimd.dma_start(warm_in[:], warm_sb[:])
        nc.gpsimd.collective_compute(
            "AllGather",
            mybir.AluOpType.bypass,
            replica_groups=[list(range(N_CORES))],
            ins=[warm_in.opt()],
            outs=[warm_out.opt()],
        )

        # resident H for both adjacencies: 64 KiB/partition each
        h_res = [
            hresp.tile([KBLK, NKG, KM * ROWS], E3, name=f"hres{a}")
            for a in range(2)
        ]

        # ---- layer 1: stream H (sync ring) + L (scalar ring), aggregate ----
        acc = psp.tile([128, MC1], F32, tag="accm", name="l1acc")
        for grp in range(NKG):
            lt = []
            for a in range(2):
                nc.sync.dma_start(
                    h_res[a][:, grp].rearrange("p (t m) -> p t m", t=KM),
                    ah[a][grp],
                )
                ltile = lringp.tile([KBLK, KM, ROWS], E4, tag="lring",
                                    name=f"l{a}_{grp}")
                nc.scalar.dma_start(ltile[:], al[a][grp])
                lt.append(ltile)
            for t in range(KM):
                kb = grp * KM + t
                # all four H matmuls, then all four L matmuls: consecutive
                # LDWEIGHTS always target a col-group whose matmul retired
                # (same-group LDW would serialize the PE pipeline)
                for a in range(2):
                    hmv = h_res[a][:, grp].rearrange(
                        "p (t m) -> p t m", t=KM
                    )
                    for mc in range(NMC1):
                        off = (a * NMC1 + mc) * 32
                        nc.tensor.matmul(
                            acc[off:off + IN_DIM, :],
                            xb[:, kb, :],
                            hmv[:, t, mc * MC1:(mc + 1) * MC1],
                            start=(kb == 0),
                            stop=False,
                            tile_position=(0, off),
                        )
                for a in range(2):
                    for mc in range(NMC1):
                        off = (a * NMC1 + mc) * 32
                        nc.tensor.matmul(
                            acc[off:off + IN_DIM, :],
                            xsb[:, kb, :],
                            lt[a][:, t, mc * MC1:(mc + 1) * MC1],
                            start=False,
                            stop=(kb == NKB - 1),
                            tile_position=(0, off),
                        )

        # ---- inter-layer: h1 = relu(z1)+relu(z2); g = h1 @ W2; exchange ----
        h1T = workp.tile([HID_DIM, ROWS], F32R)
        g_sb = workp.tile([KBLK, ROWS // KBLK, OUT_DIM], F16)
        for mc in range(NMC1):
            sl = slice(mc * MC1, (mc + 1) * MC1)
            z_ps = []
            for a in range(2):
                off = (a * NMC1 + mc) * 32
                aggc = workp.tile([IN_DIM, MC1], F32R, tag="aggc", bufs=2,
                                  name=f"aggc{a}_{mc}")
                if a == 0:
                    nc.vector.tensor_copy(aggc[:], acc[off:off + IN_DIM, :])
                else:
                    nc.scalar.activation(
                        aggc[:], acc[off:off + IN_DIM, :],
                        mybir.ActivationFunctionType.Copy,
                    )
                zp = psp.tile([HID_DIM, MC1], F32, tag="zz", bufs=2,
                              name=f"z{a}_{mc}")
                nc.tensor.matmul(zp[:], w1_sb[:], aggc[:], start=True, stop=True)
                z_ps.append(zp)
            # h1 = relu(z0 + b1) + relu(z1 + b1)
            nc.scalar.activation(h1T[:, sl], z_ps[0][:], RELU, bias=b1_sb[:])
            rtmp = workp.tile([HID_DIM, MC1], F32R, tag="rt", bufs=2,
                              name=f"rt{mc}")
            nc.vector.tensor_scalar(
                rtmp[:], z_ps[1][:], b1_sb[:], 0.0, op0=ADD, op1=MAX
            )
            nc.vector.tensor_add(h1T[:, sl], h1T[:, sl], rtmp[:])
            # own-chunk g blocks for this mc
            for j in range(mc * MC1 // KBLK, (mc + 1) * MC1 // KBLK):
                gp = psp.tile([KBLK, OUT_DIM], F32, tag="gg", bufs=2,
                              name=f"g{j}")
                nc.tensor.matmul(
                    gp[:], h1T[:, j * KBLK:(j + 1) * KBLK], w2_sb[:],
                    start=True, stop=True,
                )
                nc.vector.tensor_copy(g_sb[:, j, :], gp[:])

        # single 32 KiB store + AllGather
        g_loc = dramp.tile([KBLK, (ROWS // KBLK) * OUT_DIM], F16)
        nc.gpsimd.dma_start(g_loc[:], g_sb[:].rearrange("p j o -> p (j o)"))
        g_cat = dramp.tile([N_CORES * KBLK, (ROWS // KBLK) * OUT_DIM], F16,
                           addr_space="Shared")
        nc.gpsimd.collective_compute(
            "AllGather",
            mybir.AluOpType.bypass,
            replica_groups=[list(range(N_CORES))],
            ins=[g_loc.opt()],
            outs=[g_cat.opt()],
        )
        # rotated gather of the 7 remote chunks on the now-idle HWDGE rings
        gb2 = constp.tile([KBLK, NKB - KM, OUT_DIM], F16)
        nloc = ROWS // KBLK
        pids = {e.engine: e.partition_id() for e in (nc.sync, nc.scalar)}
        for j in range(1, N_CORES):
            eng = nc.sync if j % 2 == 1 else nc.scalar
            q = (pids[eng.engine] + j) & (N_CORES - 1)
            eng.dma_start(
                gb2[:, (j - 1) * nloc:j * nloc, :],
                g_cat[bass.ds(q * KBLK, KBLK), :]
                .rearrange("p (j2 o) -> p j2 o", j2=nloc),
            )

        def l2_lhs(s):
            return g_sb[:, s, :] if s < KM else gb2[:, s - KM, :]

        # ---- layer 2: pure PE from resident H ----
        acc2 = psp.tile([128, MC2], F32, tag="accm", name="l2acc")
        for s in range(NKB):
            grp, t = s // KM, s % KM
            for a in range(2):
                hmv = h_res[a][:, grp].rearrange("p (t m) -> p t m", t=KM)
                for mc in range(NMC2):
                    off = (a * NMC2 + mc) * 32
                    nc.tensor.matmul(
                        acc2[off:off + OUT_DIM, :],
                        l2_lhs(s),
                        hmv[:, t, mc * MC2:(mc + 1) * MC2],
                        start=(s == 0),
                        stop=(s == NKB - 1),
                        tile_position=(0, off),
                    )

        # ---- epilogue: h2 = relu(.+b2)+relu(.+b2), transpose, store ----
        h2T = workp.tile([OUT_DIM, ROWS], F32)
        o_all = workp.tile([KBLK, ROWS // KBLK, OUT_DIM], F32)
        for mc in range(NMC2):
            sl = slice(mc * MC2, (mc + 1) * MC2)
            off0 = (0 * NMC2 + mc) * 32
            off1 = (1 * NMC2 + mc) * 32
            nc.scalar.activation(
                h2T[:, sl], acc2[off0:off0 + OUT_DIM, :], RELU, bias=b2_sb[:]
            )
            r2 = workp.tile([OUT_DIM, MC2], F32, tag="r2", bufs=2,
                            name=f"r2_{mc}")
            nc.vector.tensor_scalar(
                r2[:], acc2[off1:off1 + OUT_DIM, :], b2_sb[:], 0.0,
                op0=ADD, op1=MAX,
            )
            nc.vector.tensor_add(h2T[:, sl], h2T[:, sl], r2[:])
            for j in range(mc * MC2 // KBLK, (mc + 1) * MC2 // KBLK):
                t_ps = psp.tile([KBLK, OUT_DIM], F32, tag="gg", bufs=2,
                                name=f"t{j}")
                nc.tensor.transpose(
                    t_ps[:], h2T[:, j * KBLK:(j + 1) * KBLK], ident[:]
                )
                nc.vector.tensor_copy(o_all[:, j, :], t_ps[:])
            oeng = nc.sync if mc % 2 == 0 else nc.scalar
            nblk = MC2 // KBLK
            oeng.dma_start(
                out[mc * MC2:(mc + 1) * MC2, :]
                .rearrange("(j p) o -> p j o", j=nblk),
                o_all[:, mc * nblk:(mc + 1) * nblk, :],
            )


_NC_CACHE = None


def _get_nc():
    global _NC_CACHE
    if _NC_CACHE is None:
        _NC_CACHE = _build_program()
    return _NC_CACHE


def _shard_inputs(inputs):
    E3np = ml_dtypes.float8_e3m4
    E4np = ml_dtypes.float8_e4m3
    adj = [np.asarray(inputs["adj1"], dtype=np.float32),
           np.asarray(inputs["adj2"], dtype=np.float32)]
    feat = np.asarray(inputs["features"], dtype=np.float32)
    featb0 = np.ascontiguousarray(
        feat.reshape(NKB, KBLK, IN_DIM).swapaxes(0, 1)
    )
    w1 = np.ascontiguousarray(inputs["W1"], dtype=np.float32)
    b1 = np.ascontiguousarray(inputs["b1"], dtype=np.float32).reshape(HID_DIM, 1)
    w2 = np.ascontiguousarray(inputs["W2"], dtype=np.float32)
    b2 = np.ascontiguousarray(inputs["b2"], dtype=np.float32).reshape(OUT_DIM, 1)

    # decompose once (full matrices), then per-core shard/transpose/rotate
    Hs, Ls = [], []
    for A in adj:
        H = A.astype(E3np)
        L = (16.0 * (A - H.astype(np.float32))).astype(E4np)
        Hs.append(H)
        Ls.append(L)

    in_maps = []
    for c in range(N_CORES):
        rows = slice(c * ROWS, (c + 1) * ROWS)

        def blockT(M):
            # [g, p, t, m] = M[c*ROWS + m, g*KM*128 + t*128 + p], rolled by -c
            blocked = (
                M[rows, :]
                .reshape(ROWS, NKG, KM, KBLK)
                .transpose(1, 3, 2, 0)
            )
            return np.ascontiguousarray(np.roll(blocked, -c, axis=0))

        featb = np.ascontiguousarray(np.roll(featb0, -c * ROWS // KBLK, axis=1))
        in_maps.append({
            "a1h": blockT(Hs[0]),
            "a2h": blockT(Hs[1]),
            "a1l": blockT(Ls[0]),
            "a2l": blockT(Ls[1]),
            "xb": featb.astype(np.float16),
            "xsb": (featb / 16.0).astype(np.float16),
            "w1": w1,
            "b1": b1,
            "w2": w2,
            "b2": b2,
        })
    return in_maps


def _ensure_ntff_shim():
    import sys as _sys
    try:
        import antenv.axon_hooks  # noqa: F401
    except ImportError:
        import types as _types
        mod = _types.ModuleType("antenv.axon_hooks")
        _state = {"hook": None}
        mod.set_axon_ntff_profile_hook = lambda h: _state.__setitem__("hook", h)
        mod.get_axon_ntff_profile_hook = lambda: _state["hook"]
        _sys.modules["antenv.axon_hooks"] = mod


def _run(inputs, trace=False, trace_cores=None, stitch_traces=False):
    _ensure_ntff_shim()
    nc = _get_nc()
    in_maps = _shard_inputs(inputs)
    res = run_bass_kernel_spmd(
        nc,
        in_maps,
        core_ids=list(range(N_CORES)),
        trace=trace,
        trace_cores=trace_cores,
        stitch_traces=stitch_traces,
    )
    full = np.concatenate(
        [res.results[c]["out"] for c in range(N_CORES)], axis=0
    ).astype(np.float32)
    return full, res


def kernel(**inputs):
    full, _ = _run(inputs, trace=False)
    return full


# revision 3
# speedup vs baseline: 1.0366x; 1.0366x over previous
"""Trainium2 Bass kernel v2 for the 2-layer GCN (nn_GCNClassifier).

    h1 = relu(adj1 @ x @ W1 + b1) + relu(adj2 @ x @ W1 + b1)   # [8192, 64]
    out = relu(adj1 @ g + b2) + relu(adj2 @ g + b2)            # g = h1 @ W2

Strategy (per core, 1024-row shard of both adjacencies, pre-transposed):
- Adjacency is decomposed on the host as A = H + L/16 with H = e3m4(A) and
  L = e4m3(16*(A - H)). Layer 1 streams H and L (2 bytes/elem total, same
  traffic as fp16) and consumes them with mixed-dtype matmuls: stationary
  features in fp16 (x for H, x/16 for L) x fp8 moving adjacency, both
  accumulating into the same PSUM group (unit scales).
- H stays RESIDENT in SBUF (128 KiB/partition for both adjacencies).
  Layer 2 runs entirely from the resident H with no further adjacency DMA
  (e3m4-only layer 2 measures 1.32e-2 rel err vs the f32 reference on the
  fixed test inputs; full-refinement layer 1 keeps the total there).
- Inter-layer: g = h1 @ W2 in fp16, single 32 KiB store, AllGather, then a
  7-chunk rotated gather on the (idle) HWDGE rings. Host rotates each
  core's k-group order so its own chunk is stream-position 0 (its g chunk
  feeds layer-2 matmuls before the AllGather lands).
Total HBM traffic ~33 MiB/core vs 64 MiB for the fp16 baseline.
"""

import numpy as np
import ml_dtypes

import concourse.bacc as bacc
import concourse.bass as bass
import concourse.mybir as mybir
import concourse.tile as tile
from concourse.bass_utils import run_bass_kernel_spmd
from concourse.masks import make_identity

N = 8192
IN_DIM, HID_DIM, OUT_DIM = 32, 64, 16
N_CORES = 8
ROWS = N // N_CORES          # 1024 output rows per core
KBLK = 128                   # contraction block (SBUF partition dim)
KM = 8                       # k-blocks per DMA group (1 MiB fp8 groups)
NKB = N // KBLK              # 64 contraction blocks
NKG = NKB // KM              # 8 groups per adjacency
MC1 = 512                    # layer-1 moving chunk
MC2 = 512                    # layer-2 moving chunk
F32 = mybir.dt.float32
F32R = mybir.dt.float32r    # single-pass PE fp32 (vs 4-pass full fp32)
F16 = mybir.dt.float16
E3 = mybir.dt.float8e3      # H: e3m4
E4 = mybir.dt.float8e4      # L: e4m3 (16x residual)
RELU = mybir.ActivationFunctionType.Relu
ADD = mybir.AluOpType.add
MAX = mybir.AluOpType.mult if False else mybir.AluOpType.max
LRING = 4                    # L-group ring depth


def _build_program():
    nc = bacc.Bacc(
        "TRN2", target_bir_lowering=False, debug=False, num_devices=N_CORES
    )
    a1h = nc.dram_tensor("a1h", [NKG, KBLK, KM, ROWS], E3, kind="ExternalInput")
    a2h = nc.dram_tensor("a2h", [NKG, KBLK, KM, ROWS], E3, kind="ExternalInput")
    a1l = nc.dram_tensor("a1l", [NKG, KBLK, KM, ROWS], E4, kind="ExternalInput")
    a2l = nc.dram_tensor("a2l", [NKG, KBLK, KM, ROWS], E4, kind="ExternalInput")
    xb = nc.dram_tensor("xb", [KBLK, NKB, IN_DIM], F16, kind="ExternalInput")
    xsb = nc.dram_tensor("xsb", [KBLK, NKB, IN_DIM], F16, kind="ExternalInput")
    w1 = nc.dram_tensor("w1", [IN_DIM, HID_DIM], F32, kind="ExternalInput")
    b1 = nc.dram_tensor("b1", [HID_DIM, 1], F32, kind="ExternalInput")
    w2 = nc.dram_tensor("w2", [HID_DIM, OUT_DIM], F32, kind="ExternalInput")
    b2 = nc.dram_tensor("b2", [OUT_DIM, 1], F32, kind="ExternalInput")
    out = nc.dram_tensor("out", [ROWS, OUT_DIM], F32, kind="ExternalOutput")

    with tile.TileContext(nc) as tc:
        _kernel_body(nc, tc, (a1h, a2h), (a1l, a2l), xb, xsb, w1, b1, w2, b2, out)
    nc.compile()
    return nc


def _kernel_body(nc, tc, ah, al, xb_d, xsb_d, w1, b1, w2, b2, out):
    NMC1 = ROWS // MC1
    NMC2 = ROWS // MC2
    with (
        tc.tile_pool(name="const", bufs=1) as constp,
        tc.tile_pool(name="hres", bufs=1) as hresp,
        tc.tile_pool(name="lring", bufs=LRING) as lringp,
        tc.tile_pool(name="work", bufs=1) as workp,
        tc.tile_pool(name="psum", bufs=1, space="PSUM") as psp,
        tc.tile_pool(name="dram", bufs=1, space="DRAM") as dramp,
    ):
        # features (fp16) lead the two HWDGE rings; constants ride SWDGE
        xb = constp.tile([KBLK, NKB, IN_DIM], F16)
        nc.sync.dma_start(xb[:], xb_d[:])
        xsb = constp.tile([KBLK, NKB, IN_DIM], F16)
        nc.scalar.dma_start(xsb[:], xsb_d[:])
        w1_sb = constp.tile([IN_DIM, HID_DIM], F32R)
        nc.gpsimd.dma_start(w1_sb[:], w1[:])
        b1_sb = constp.tile([HID_DIM, 1], F32)
        nc.gpsimd.dma_start(b1_sb[:], b1[:])
        w2_sb = constp.tile([HID_DIM, OUT_DIM], F32R)
        nc.gpsimd.dma_start(w2_sb[:], w2[:])
        b2_sb = constp.tile([OUT_DIM, 1], F32)
        nc.gpsimd.dma_start(b2_sb[:], b2[:])
        ident = constp.tile([OUT_DIM, OUT_DIM], F32)
        make_identity(nc, ident[:])

        # tiny warm-up AllGather to wake the CC path early
        warm_sb = constp.tile([1, N_CORES], F32)
        nc.gpsimd.memset(warm_sb[:], 0.0)
        warm_in = dramp.tile([1, N_CORES], F32)
        warm_out = dramp.tile([N_CORES, N_CORES], F32, addr_space="Shared")
        nc.gpsimd.dma_start(warm_in[:], warm_sb[:])
        nc.gpsimd.collective_compute(
            "AllGather",
            mybir.AluOpType.bypass,
            replica_groups=[list(range(N_CORES))],
            ins=[warm_in.opt()],
            outs=[warm_out.opt()],
        )

        # resident H for both adjacencies: 64 KiB/partition each
        h_res = [
            hresp.tile([KBLK, NKG, KM * ROWS], E3, name=f"hres{a}")
            for a in range(2)
        ]

        # ---- layer 1: stream H (sync ring) + L (scalar ring), aggregate ----
        acc = psp.tile([128, MC1], F32, tag="accm", name="l1acc")
        for grp in range(NKG):
            lt = []
            for a in range(2):
                nc.sync.dma_start(
                    h_res[a][:, grp].rearrange("p (t m) -> p t m", t=KM),
                    ah[a][grp],
                )
                ltile = lringp.tile([KBLK, KM, ROWS], E4, tag="lring",
                                    name=f"l{a}_{grp}")
                nc.scalar.dma_start(ltile[:], al[a][grp])
                lt.append(ltile)
            for t in range(KM):
                kb = grp * KM + t
                # all four H matmuls, then all four L matmuls: consecutive
                # LDWEIGHTS always target a col-group whose matmul retired
                # (same-group LDW would serialize the PE pipeline)
                for a in range(2):
                    hmv = h_res[a][:, grp].rearrange(
                        "p (t m) -> p t m", t=KM
                    )
                    for mc in range(NMC1):
                        off = (a * NMC1 + mc) * 32
                        nc.tensor.matmul(
                            acc[off:off + IN_DIM, :],
                            xb[:, kb, :],
                            hmv[:, t, mc * MC1:(mc + 1) * MC1],
                            start=(kb == 0),
                            stop=False,
                            tile_position=(0, off),
                        )
                for a in range(2):
                    for mc in range(NMC1):
                        off = (a * NMC1 + mc) * 32
                        nc.tensor.matmul(
                            acc[off:off + IN_DIM, :],
                            xsb[:, kb, :],
                            lt[a][:, t, mc * MC1:(mc + 1) * MC1],
                            start=False,
                            stop=(kb == NKB - 1),
                            tile_position=(0, off),
                        )

        # ---- inter-layer: h1 = relu(z1)+relu(z2); g = h1 @ W2; exchange ----
        h1T = workp.tile([HID_DIM, ROWS], F32R)
        g_sb = workp.tile([KBLK, ROWS // KBLK, OUT_DIM], F16)
        g_loc = dramp.tile([KBLK, (ROWS // KBLK) * OUT_DIM], F16)
        gflat = g_sb[:].rearrange("p j o -> p (j o)")
        for mc in range(NMC1):
            sl = slice(mc * MC1, (mc + 1) * MC1)
            z_ps = []
            for a in range(2):
                off = (a * NMC1 + mc) * 32
                aggc = workp.tile([IN_DIM, MC1], F32R, tag="aggc", bufs=2,
                                  name=f"aggc{a}_{mc}")
                if a == 0:
                    nc.vector.tensor_copy(aggc[:], acc[off:off + IN_DIM, :])
                else:
                    nc.scalar.activation(
                        aggc[:], acc[off:off + IN_DIM, :],
                        mybir.ActivationFunctionType.Copy,
                    )
                zp = psp.tile([HID_DIM, MC1], F32, tag="zz", bufs=2,
                              name=f"z{a}_{mc}")
                nc.tensor.matmul(zp[:], w1_sb[:], aggc[:], start=True, stop=True)
                z_ps.append(zp)
            # h1 = relu(z0 + b1) + relu(z1 + b1)
            nc.scalar.activation(h1T[:, sl], z_ps[0][:], RELU, bias=b1_sb[:])
            rtmp = workp.tile([HID_DIM, MC1], F32R, tag="rt", bufs=2,
                              name=f"rt{mc}")
            nc.scalar.activation(rtmp[:], z_ps[1][:], RELU, bias=b1_sb[:])
            nc.vector.tensor_add(h1T[:, sl], h1T[:, sl], rtmp[:])
            # own-chunk g blocks for this mc; casts on scalar (DVE is the
            # chain bottleneck), half-store as soon as this mc's blocks land
            for j in range(mc * MC1 // KBLK, (mc + 1) * MC1 // KBLK):
                gp = psp.tile([KBLK, OUT_DIM], F32, tag="gg", bufs=2,
                              name=f"g{j}")
                nc.tensor.matmul(
                    gp[:], h1T[:, j * KBLK:(j + 1) * KBLK], w2_sb[:],
                    start=True, stop=True,
                )
                nc.scalar.activation(
                    g_sb[:, j, :], gp[:], mybir.ActivationFunctionType.Copy
                )
            half = (ROWS // KBLK) // NMC1 * OUT_DIM
            nc.gpsimd.dma_start(
                g_loc[:, mc * half:(mc + 1) * half],
                gflat[:, mc * half:(mc + 1) * half],
            )

        g_cat = dramp.tile([N_CORES * KBLK, (ROWS // KBLK) * OUT_DIM], F16,
                           addr_space="Shared")
        nc.gpsimd.collective_compute(
            "AllGather",
            mybir.AluOpType.bypass,
            replica_groups=[list(range(N_CORES))],
            ins=[g_loc.opt()],
            outs=[g_cat.opt()],
        )
        # rotated gather of the 7 remote chunks on the now-idle HWDGE rings
        gb2 = constp.tile([KBLK, NKB - KM, OUT_DIM], F16)
        nloc = ROWS // KBLK
        pids = {e.engine: e.partition_id() for e in (nc.sync, nc.scalar)}
        for j in range(1, N_CORES):
            eng = nc.sync if j % 2 == 1 else nc.scalar
            q = (pids[eng.engine] + j) & (N_CORES - 1)
            eng.dma_start(
                gb2[:, (j - 1) * nloc:j * nloc, :],
                g_cat[bass.ds(q * KBLK, KBLK), :]
                .rearrange("p (j2 o) -> p j2 o", j2=nloc),
            )

        def l2_lhs(s):
            return g_sb[:, s, :] if s < KM else gb2[:, s - KM, :]

        # ---- layer 2: pure PE from resident H ----
        acc2 = psp.tile([128, MC2], F32, tag="accm", name="l2acc")
        for s in range(NKB):
            grp, t = s // KM, s % KM
            for a in range(2):
                hmv = h_res[a][:, grp].rearrange("p (t m) -> p t m", t=KM)
                for mc in range(NMC2):
                    off = (a * NMC2 + mc) * 32
                    nc.tensor.matmul(
                        acc2[off:off + OUT_DIM, :],
                        l2_lhs(s),
                        hmv[:, t, mc * MC2:(mc + 1) * MC2],
                        start=(s == 0),
                        stop=(s == NKB - 1),
                        tile_position=(0, off),
                    )

        # ---- epilogue: h2 = relu(.+b2)+relu(.+b2), transpose, store ----
        h2T = workp.tile([OUT_DIM, ROWS], F32)
        o_all = workp.tile([KBLK, ROWS // KBLK, OUT_DIM], F32)
        for mc in range(NMC2):
            sl = slice(mc * MC2, (mc + 1) * MC2)
            off0 = (0 * NMC2 + mc) * 32
            off1 = (1 * NMC2 + mc) * 32
            nc.scalar.activation(
                h2T[:, sl], acc2[off0:off0 + OUT_DIM, :], RELU, bias=b2_sb[:]
            )
            r2 = workp.tile([OUT_DIM, MC2], F32, tag="r2", bufs=2,
                            name=f"r2_{mc}")
            nc.vector.tensor_scalar(
                r2[:], acc2[off1:off1 + OUT_DIM, :], b2_sb[:], 0.0,
                op0=ADD, op1=MAX,
            )
            nc.vector.tensor_add(h2T[:, sl], h2T[:, sl], r2[:])
            for j in range(mc * MC2 // KBLK, (mc + 1) * MC2 // KBLK):
                t_ps = psp.tile([KBLK, OUT_DIM], F32, tag="gg", bufs=2,
                                name=f"t{j}")
                nc.tensor.transpose(
                    t_ps[:], h2T[:, j * KBLK:(j + 1) * KBLK], ident[:]
                )
                nc.vector.tensor_copy(o_all[:, j, :], t_ps[:])
            oeng = nc.sync if mc % 2 == 0 else nc.scalar
            nblk = MC2 // KBLK
            oeng.dma_start(
                out[mc * MC2:(mc + 1) * MC2, :]
                .rearrange("(j p) o -> p j o", j=nblk),
                o_all[:, mc * nblk:(mc + 1) * nblk, :],
            )


_NC_CACHE = None


def _get_nc():
    global _NC_CACHE
    if _NC_CACHE is None:
        _NC_CACHE = _build_program()
    return _NC_CACHE


def _shard_inputs(inputs):
    E3np = ml_dtypes.float8_e3m4
    E4np = ml_dtypes.float8_e4m3
    adj = [np.asarray(inputs["adj1"], dtype=np.float32),
           np.asarray(inputs["adj2"], dtype=np.float32)]
    feat = np.asarray(inputs["features"], dtype=np.float32)
    featb0 = np.ascontiguousarray(
        feat.reshape(NKB, KBLK, IN_DIM).swapaxes(0, 1)
    )
    w1 = np.ascontiguousarray(inputs["W1"], dtype=np.float32)
    b1 = np.ascontiguousarray(inputs["b1"], dtype=np.float32).reshape(HID_DIM, 1)
    w2 = np.ascontiguousarray(inputs["W2"], dtype=np.float32)
    b2 = np.ascontiguousarray(inputs["b2"], dtype=np.float32).reshape(OUT_DIM, 1)

    # decompose once (full matrices), then per-core shard/transpose/rotate
    Hs, Ls = [], []
    for A in adj:
        H = A.astype(E3np)
        L = (16.0 * (A - H.astype(np.float32))).astype(E4np)
        Hs.append(H)
        Ls.append(L)

    in_maps = []
    for c in range(N_CORES):
        rows = slice(c * ROWS, (c + 1) * ROWS)

        def blockT(M):
            # [g, p, t, m] = M[c*ROWS + m, g*KM*128 + t*128 + p], rolled by -c
            blocked = (
                M[rows, :]
                .reshape(ROWS, NKG, KM, KBLK)
                .transpose(1, 3, 2, 0)
            )
            return np.ascontiguousarray(np.roll(blocked, -c, axis=0))

        featb = np.ascontiguousarray(np.roll(featb0, -c * ROWS // KBLK, axis=1))
        in_maps.append({
            "a1h": blockT(Hs[0]),
            "a2h": blockT(Hs[1]),
            "a1l": blockT(Ls[0]),
            "a2l": blockT(Ls[1]),
            "xb": featb.astype(np.float16),
            "xsb": (featb / 16.0).astype(np.float16),
            "w1": w1,
            "b1": b1,
            "w2": w2,
            "b2": b2,
        })
    return in_maps


def _ensure_ntff_shim():
    import sys as _sys
    try:
        import antenv.axon_hooks  # noqa: F401
    except ImportError:
        import types as _types
        mod = _types.ModuleType("antenv.axon_hooks")
        _state = {"hook": None}
        mod.set_axon_ntff_profile_hook = lambda h: _state.__setitem__("hook", h)
        mod.get_axon_ntff_profile_hook = lambda: _state["hook"]
        _sys.modules["antenv.axon_hooks"] = mod


def _run(inputs, trace=False, trace_cores=None, stitch_traces=False):
    _ensure_ntff_shim()
    nc = _get_nc()
    in_maps = _shard_inputs(inputs)
    res = run_bass_kernel_spmd(
        nc,
        in_maps,
        core_ids=list(range(N_CORES)),
        trace=trace,
        trace_cores=trace_cores,
        stitch_traces=stitch_traces,
    )
    full = np.concatenate(
        [res.results[c]["out"] for c in range(N_CORES)], axis=0
    ).astype(np.float32)
    return full, res


def kernel(**inputs):
    full, _ = _run(inputs, trace=False)
    return full
